# revision 1
# baseline (speedup 1.0000x reference)
"""Trainium2 Bass kernel for nn_MergeNN (retrieval_knn), 8 NeuronCores.

Sharding: the N=20000 reference-dataset axis is split 2500/core (padded to
2560 = 20 tiles of 128). Each core computes its [N/8, B] kernel slices fully
fused; partial sums are AllReduced (bf16 after phase 1, fp32 per branch after
phase 2) and every core finishes with the identical [32, B] output.

v6 design:
- All static operands are host-precomputed and DMAd once into SBUF
  residents; exp row-biases carry the -1e30 pad kill.
- Bulk dist/consume matmuls run in bf16 (1 col/cycle on the PE vs ~1.5 for
  fp32r); the y/argmin/broadcast matmul path stays fp32r.
- ldist is factored on the host as a rank-64 SVD (tail residual ~0.09 on a
  [0,1] matrix; x ETA = 9e-4 in the exponent). The label-distance term
  -ETA*ldist[lidx[n], yidx[q]] then folds into the SAME K=128 distance
  matmul as the features: lhsT rows = [fT; A[lidx].T], rhs rows =
  [xt; (-ETA/2) B^T onehot], halving phase-2 matmul count.
- Exactly three collectives (each pays cross-core skew): AR1 (bf16, after
  phase 1), AR2(b0) (hidden under P2(b1)), AR2(b1) (exposed tail).
- argmin one-hot = (d == rowmin), PE-transposed to [L, B] (exact-tie
  deviation from first-index semantics is measure-zero and bounded).
- Reciprocals on DVE in [128, k] layout (cost ~ free size) with a DRAM
  round-trip back to a [1, B] row.
- e_acc accumulation split DVE / GPSIMD; esum via ones-matmul.
- exp columns are only used in num/den ratios, so per-query exponent
  factors cancel and are dropped.
"""
import contextlib
import sys

sys.path.insert(0, "/opt/trn_rl_repo")

import ml_dtypes
import numpy as np

import concourse.bacc as bacc
import concourse.tile as tile
from concourse import mybir
from concourse.alu_op_type import AluOpType
from concourse.bass_utils import run_bass_kernel_spmd

F32 = mybir.dt.float32
F32R = mybir.dt.float32r
BF16 = mybir.dt.bfloat16
AF = mybir.ActivationFunctionType
AX = mybir.AxisListType

NCORES = 8
N, B, D, DY, L = 20000, 2048, 64, 32, 100
ETA = 0.01
RK = 64                          # ldist SVD rank kept
NSH_RAW = N // NCORES            # 2500
NT = (NSH_RAW + 127) // 128      # 20
NSH = NT * 128                   # 2560
NK = B // 128                    # 16
NB4 = B // 512                   # 4
HB = B // 2                      # P1 half width
HS = HB // 512
AR1_DT = BF16
AR2_DT = F32


def build_nc(n_cores=NCORES):
    nc = bacc.Bacc("TRN2", target_bir_lowering=False, debug=False,
                   enable_asserts=False, num_devices=n_cores)
    I = {}
    for name, shape, dt_ in [
        ("xT", [128, B], BF16),     # rows D:128 zero (K=128 streams faster)
        ("sfT", [128, NSH], BF16),  # rows D:128 zero
        ("f12t", [128, NT * 128], BF16),      # P1 consume lhsT tiles
        ("fA1", [128, NSH], BF16),            # [f_jT ; A_j[lidx].T]
        ("fA2", [128, NSH], BF16),
        ("slo", [128, NT * (DY + 1)], BF16),  # labels+ones consume tiles
        ("negnS", [128, NT], F32), ("negn1", [128, NT], F32),
        ("negn2", [128, NT], F32),
        ("uqr2d", [128, 256], F32R),  # blockdiag uqr pair (rows 0/64)
        ("Wb1", [D + 1, DY + 1], F32R), ("Wb2", [D + 1, DY + 1], F32R),
        ("Bsc1", [L, RK], F32R), ("Bsc2", [L, RK], F32R),
        ("ident", [128, 128], F32), ("onesr", [1, 128], F32R),
        ("onesc", [128, 1], BF16),
    ]:
        I[name] = nc.dram_tensor(name, shape, dt_, kind="ExternalInput").ap()
    outT_ap = nc.dram_tensor("outT", [DY, B], F32, kind="ExternalOutput").ap()

    with tile.TileContext(nc) as tc:
        kernel_body(tc, I, outT_ap, n_cores=n_cores)
    nc.compile()
    return nc


def kernel_body(tc, I, outT_ap, *, n_cores):
    nc = tc.nc
    groups = [list(range(n_cores))]
    ctx = contextlib.ExitStack()
    with ctx:
        const = ctx.enter_context(tc.tile_pool(name="const", bufs=1))
        dram = ctx.enter_context(tc.tile_pool(name="dram", bufs=1,
                                              space="DRAM"))

        R = {}

        def load(pool, names):
            for name in names:
                t = pool.tile(list(I[name].shape), I[name].dtype, tag=name,
                              name=name)
                nc.sync.dma_start(t, I[name])
                R[name] = t

        load(const, ["negnS", "negn1", "negn2", "onesc"])
        # xg rows 0:64 = xt (bf16), rows 64:128 = (-ETA/2) B^T onehot
        xg = [const.tile([128, B], BF16, tag=f"xg{j}", name=f"xg{j}")
              for j in (0, 1)]
        e_acc = const.tile([128, B], F32, tag="e_acc", name="e_acc")
        nc.vector.memset(e_acc, 0.0)
        stgA = ctx.enter_context(tc.tile_pool(name="stgA", bufs=1))
        stgC = ctx.enter_context(tc.tile_pool(name="stgC", bufs=1))

        # P1-only residents in a stack-scoped pool, released after phase 1
        # so the phase-2 residents + interlude tiles reuse the space.
        p1c = tc.alloc_tile_pool(name="p1c", bufs=1)
        for name in ("xT", "sfT", "f12t"):
            t = p1c.tile(list(I[name].shape), I[name].dtype, tag=name,
                         name=name)
            R[name] = t
        half = NT // 2 * 128
        nc.sync.dma_start(R["xT"][:, 0:HB], I["xT"][:, 0:HB])
        nc.sync.dma_start(R["sfT"][:, 0:half], I["sfT"][:, 0:half])
        nc.sync.dma_start(R["f12t"][:, 0:half], I["f12t"][:, 0:half])
        nc.sync.dma_start(R["sfT"][:, half:], I["sfT"][:, half:])
        nc.sync.dma_start(R["f12t"][:, half:], I["f12t"][:, half:])
        nc.sync.dma_start(R["xT"][:, HB:], I["xT"][:, HB:])

        # DVE reciprocal cost ~ free size: invert the [1, w] den row as
        # [128, w/128] (read from the collective's DRAM output), then
        # round-trip to a [1, w] SBUF row for the broadcast matmul.
        def make_recip(pool, dram_row, rcp_row, tag, w, scale=None):
            k = w // 128
            den16 = pool.tile([128, k], dram_row.dtype, tag=f"d16{tag}",
                              name=f"d16{tag}")
            nc.sync.dma_start(
                den16, dram_row.rearrange("a (p k) -> (a p) k", k=k))
            rcp16 = pool.tile([128, k], F32R, tag=f"r16{tag}",
                              name=f"r16{tag}")
            with nc.allow_low_precision(
                    reason="fp32r recip feeds fp32r broadcast matmul"):
                nc.vector.reciprocal(rcp16, den16)
            if scale is not None:
                nc.vector.tensor_scalar(rcp16, rcp16, scale, None,
                                        AluOpType.mult)
            drcp = dram.tile([1, w], F32R, tag=f"drcp{tag}", name=f"drcp{tag}")
            nc.sync.dma_start(
                drcp.rearrange("a (p k) -> (a p) k", k=k), rcp16)
            nc.sync.dma_start(rcp_row, drcp)

        # CC warm-up: a tiny collective issued first so the CC core's
        # cold-start and cross-core launch skew are absorbed during P1
        # (the first real collective then starts ~1 us after trigger).
        wu_i = dram.tile([1, 128], F32, tag="wui", name="wui")
        wu_o = dram.tile([1, 128], F32, tag="wuo", name="wuo",
                         addr_space="Shared")
        nc.vector.memset(wu_sb := stgA.tile([1, 128], F32, tag="wusb",
                                            name="wusb"), 0.0)
        nc.sync.dma_start(wu_i, wu_sb)
        nc.gpsimd.collective_compute(
            "AllReduce", AluOpType.add, replica_groups=groups,
            ins=[wu_i.opt()], outs=[wu_o.opt()])

        # ========== phase 1: two half-width passes, one AllReduce ==========
        with tc.tile_pool(name="acc12p", bufs=1, space="PSUM") as accp:
            acc12 = accp.tile([128, B], F32, tag="acc12")
            for h in (0, 1):
                c0 = h * HB
                with (
                    tc.tile_pool(name=f"pdp{h}", bufs=2, space="PSUM") as pdp,
                    tc.tile_pool(name=f"ep{h}", bufs=3) as ep,
                ):
                    def consume1(pe, pi):
                        lhs_c = R["f12t"][:, pi * 128:(pi + 1) * 128]
                        for q in range(HS):
                            nc.tensor.matmul(
                                acc12[:, c0 + q * 512:c0 + (q + 1) * 512],
                                lhs_c, pe[:, q * 512:(q + 1) * 512],
                                start=(pi == 0), stop=(pi == NT - 1))

                    prev = None
                    for i in range(NT):
                        r0 = i * 128
                        pd = pdp.tile([128, HB], F32, tag="pd")
                        lhs_d = R["sfT"][:, r0:r0 + 128]
                        for q in range(HS):
                            nc.tensor.matmul(
                                pd[:, q * 512:(q + 1) * 512], lhs_d,
                                R["xT"][:, c0 + q * 512:c0 + (q + 1) * 512],
                                start=True, stop=True)
                        e_t = ep.tile([128, HB], BF16, tag="e")
                        nc.scalar.activation(e_t, pd, AF.Exp,
                                             bias=R["negnS"][:, i:i + 1],
                                             scale=2.0)
                        nc.vector.tensor_tensor(
                            e_acc[:, c0:c0 + HB], e_acc[:, c0:c0 + HB],
                            e_t, AluOpType.add)
                        if prev is not None:
                            consume1(*prev)
                        prev = (e_t, i)
                    consume1(*prev)
                if h == 0:
                    st1n = stgA.tile([2 * D, B], AR1_DT, tag="st1n",
                                     name="st1n")
                    nc.vector.tensor_copy(st1n[:, 0:HB], acc12[:, 0:HB])

            p1c.release()
            p2c = tc.alloc_tile_pool(name="p2c", bufs=1)
            itl = tc.alloc_tile_pool(name="itl", bufs=1)
            xt = [itl.tile([D + 1, B], F32R, tag=f"xtj{j}", name=f"xtj{j}")
                  for j in (0, 1)]
            for j in (0, 1):
                nc.vector.memset(xt[j][D:D + 1, :].bitcast(F32), 1.0)
            ylh2 = itl.tile([128, B], F32R, tag="ylh2", name="ylh2")
            nc.vector.memset(ylh2.bitcast(F32), 0.0)

            # esum + stage + single AR1 (h0's columns staged during the
            # h1 pass via the emission point below)
            st1d = stgA.tile([1, B], AR1_DT, tag="st1d", name="st1d")
            with tc.tile_pool(name="esp", bufs=1, space="PSUM") as esp:
                e_accR = stgA.tile([128, B], BF16, tag="e_accR",
                                   name="e_accR")
                nc.scalar.copy(e_accR, e_acc)
                esum = esp.tile([1, B], F32, tag="esum")
                for q in range(NB4):
                    nc.tensor.matmul(esum[:, q * 512:(q + 1) * 512],
                                     R["onesc"],
                                     e_accR[:, q * 512:(q + 1) * 512],
                                     start=True, stop=True)
                nc.vector.tensor_copy(st1n[:, HB:], acc12[:, HB:])
                nc.vector.tensor_copy(st1d, esum)
        ar1_in = dram.tile([2 * D + 1, B], AR1_DT, tag="ar1i", name="ar1i")
        ar1_out = dram.tile([2 * D + 1, B], AR1_DT, tag="ar1o", name="ar1o",
                            addr_space="Shared")
        nc.sync.dma_start(ar1_in[0:2 * D, :], st1n)
        nc.sync.dma_start(ar1_in[2 * D:2 * D + 1, :], st1d)
        nc.gpsimd.collective_compute(
            "AllReduce", AluOpType.add, replica_groups=groups,
            ins=[ar1_in.opt()], outs=[ar1_out.opt()])
        # phase-2 residents load while AR1 is in flight (emitted after the
        # staging DMAs so they don't delay the collective trigger)
        load(p2c, ["fA1", "fA2", "slo", "uqr2d", "Wb1", "Wb2",
                   "Bsc1", "Bsc2", "ident", "onesr"])

        # ============== xt build ==============
        arb = stgA.tile([2 * D, B], AR1_DT, tag="arb", name="arb")
        nc.sync.dma_start(arb, ar1_out[0:2 * D, :])
        rcp = stgA.tile([1, B], F32R, tag="rcp", name="rcp")
        make_recip(stgA, ar1_out[2 * D:2 * D + 1, :], rcp, "a", B)
        with tc.tile_pool(name="bcp", bufs=1, space="PSUM") as bcp:
            bc = bcp.tile([128, B], F32, tag="bc")
            for q in range(NB4):
                nc.tensor.matmul(bc[:, q * 512:(q + 1) * 512], R["onesr"],
                                 rcp[:, q * 512:(q + 1) * 512],
                                 start=True, stop=True)
            nc.vector.tensor_tensor(xt[0][0:D, :], arb[0:D, :], bc[0:D, :],
                                    AluOpType.mult)
            nc.vector.tensor_tensor(xt[1][0:D, :], arb[D:2 * D, :],
                                    bc[D:2 * D, :], AluOpType.mult)
        for j in (0, 1):
            nc.scalar.copy(xg[j][0:D, :], xt[j][0:D, :])

        # ============== interlude per branch ==============
        # ylh -> label distances -> argmin one-hot -> PE-transpose ->
        # xg rows 64:128 = Bsc^T @ onehot
        # ylh for both branches into one [2*(DY+1), B] stage; the label
        # distances for BOTH branches then come from one matmul per chunk
        # (lhsT = stacked ylh pair, rhs = block-diagonal uqr pair).
        oh, vt_sb = {}, {}
        # ylh pair at partition bases 0 and 64; gap rows are zeroed once at
        # start (ylh2_zero) so the K=128 dps lhsT contracts clean zeros.
        with tc.tile_pool(name="ips", bufs=1, space="PSUM") as ips:
            ylh_ps = {j: ips.tile([DY + 1, B], F32, tag=f"ylh{j}",
                                  name=f"ylhp{j}") for j in (0, 1)}
            for j in (0, 1):
                for q in range(NB4):
                    nc.tensor.matmul(
                        ylh_ps[j][:, q * 512:(q + 1) * 512],
                        R[f"Wb{j+1}"],
                        xt[j][:, q * 512:(q + 1) * 512],
                        start=True, stop=True)
            # branch 0 lands on partitions 0:33 directly; branch 1 needs a
            # partition shift to 64:97, which only DMA can do
            nc.scalar.copy(ylh2[0:DY + 1, :], ylh_ps[0])
            ylhs1 = itl.tile([DY + 1, B], F32R, tag="ylhs1", name="ylhs1")
            nc.scalar.copy(ylhs1, ylh_ps[1])
            nc.sync.dma_start(ylh2[64:64 + DY + 1, :], ylhs1)
        with tc.tile_pool(name="dps", bufs=1, space="PSUM") as dpp:
            dps = dpp.tile([128, NK * 256], F32, tag="dps")
            for k in range(NK):
                nc.tensor.matmul(dps[:, k * 256:(k + 1) * 256],
                                 ylh2[:, k * 128:(k + 1) * 128],
                                 R["uqr2d"], start=True, stop=True)
            d4 = dps.rearrange("p (k j l) -> p k j l", j=2, l=128)
            for j in (0, 1):
                d3 = d4[:, :, j, 0:L]
                dmin = itl.tile([128, NK], F32, tag=f"dmin{j}",
                                 name=f"dmin{j}")
                nc.vector.tensor_reduce(dmin, d3, AX.X, AluOpType.min)
                # argmin one-hot = (d == rowmin); exact-tie deviation from
                # the reference's first-index pick is measure-zero, bounded.
                oh[j] = itl.tile([128, NK * L], F32, tag=f"ohs{j}",
                                  name=f"ohs{j}")
                oh3 = oh[j].rearrange("p (k l) -> p k l", l=L)
                nc.vector.tensor_tensor(
                    oh3, d3, dmin[:, :, None].broadcast_to((128, NK, L)),
                    AluOpType.is_equal)
        with tc.tile_pool(name="vtp", bufs=1, space="PSUM") as vtp:
            for j in (0, 1):
                vt_ps = vtp.tile([L, B], F32, tag=f"vt{j}")
                oh3 = oh[j].rearrange("p (k l) -> p k l", l=L)
                for k in range(NK):
                    nc.tensor.transpose(vt_ps[:, k * 128:(k + 1) * 128],
                                        oh3[:, k, :], R["ident"])
                vt_sb[j] = itl.tile([L, B], F32R, tag=f"vts{j}",
                                     name=f"vts{j}")
                nc.scalar.copy(vt_sb[j], vt_ps)
        with tc.tile_pool(name="bhp", bufs=1, space="PSUM") as bhp:
            for j in (0, 1):
                bh_ps = bhp.tile([RK, B], F32, tag=f"bh{j}")
                for q in range(NB4):
                    nc.tensor.matmul(bh_ps[:, q * 512:(q + 1) * 512],
                                     R[f"Bsc{j+1}"],
                                     vt_sb[j][:, q * 512:(q + 1) * 512],
                                     start=True, stop=True)
                nc.scalar.copy(xg[j][D:D + RK, :], bh_ps)

        itl.release()

        # ============== phase 2 per branch: K=128 fused dist ==============
        def p2_branch(j, acc2):
            negn = R[f"negn{j+1}"]
            fA = R[f"fA{j+1}"]
            with (
                tc.tile_pool(name=f"pd2p{j}", bufs=2, space="PSUM") as pdp,
                tc.tile_pool(name=f"e2p{j}", bufs=3) as e2p,
            ):
                def consume2(pes, pi):
                    lhs_s = R["slo"][:, pi * (DY + 1):(pi + 1) * (DY + 1)]
                    for c in range(2):
                        for q in range(HS):
                            col = c * 1024 + q * 512
                            nc.tensor.matmul(
                                acc2[:, col:col + 512], lhs_s,
                                pes[c][:, q * 512:(q + 1) * 512],
                                start=(pi == 0), stop=(pi == NT - 1))

                prev = None
                for i in range(NT):
                    r0 = i * 128
                    lhs_f = fA[:, r0:r0 + 128]
                    pes = []
                    for c in range(2):
                        pd2 = pdp.tile([128, HB], F32, tag="pd2")
                        for q in range(HS):
                            col = c * 1024 + q * 512
                            nc.tensor.matmul(
                                pd2[:, q * 512:(q + 1) * 512], lhs_f,
                                xg[j][:, col:col + 512],
                                start=True, stop=True)
                        e2 = e2p.tile([128, HB], BF16, tag="e2")
                        nc.scalar.activation(e2, pd2, AF.Exp,
                                             bias=negn[:, i:i + 1],
                                             scale=2.0)
                        pes.append(e2)
                    if prev is not None:
                        consume2(*prev)
                    prev = (pes, i)
                consume2(*prev)

        # finish: y = num * (0.5/den) for one AR2 output
        def finish(ar_out, tag):
            rcp2 = stgC.tile([1, B], F32R, tag="rcp2", name=f"rcp2{tag}")
            make_recip(stgC, ar_out[DY:DY + 1, :], rcp2, "b", B,
                       scale=0.5)
            aro2 = stgC.tile([DY, B], AR2_DT, tag="aro2",
                             name=f"aro2{tag}")
            nc.sync.dma_start(aro2, ar_out[0:DY, :])
            y = stgC.tile([DY, B], F32R, tag=f"y{tag}", name=f"y{tag}")
            nc.gpsimd.partition_broadcast(y, rcp2)
            nc.vector.tensor_tensor(y, aro2, y, AluOpType.mult)
            return y

        ar2_i, ar2_o = {}, {}
        for j in (0, 1):
            ar2_i[j] = dram.tile([DY + 1, B], AR2_DT, tag=f"ar2i{j}",
                                 name=f"ar2i{j}")
            ar2_o[j] = dram.tile([DY + 1, B], AR2_DT, tag=f"ar2o{j}",
                                 name=f"ar2o{j}", addr_space="Shared")
        y0 = None
        for j in (0, 1):
            st2 = stgC.tile([DY + 1, B], AR2_DT, tag="st2",
                            name=f"st2_{j}")
            with tc.tile_pool(name=f"acc2p{j}", bufs=1, space="PSUM") as a2p:
                acc2 = a2p.tile([DY + 1, B], F32, tag="acc2")
                p2_branch(j, acc2)
                nc.vector.tensor_copy(st2, acc2)
            nc.sync.dma_start(ar2_i[j], st2)
            nc.gpsimd.collective_compute(
                "AllReduce", AluOpType.add, replica_groups=groups,
                ins=[ar2_i[j].opt()], outs=[ar2_o[j].opt()])
            if j == 0:
                # b0's finish is emitted here so it hides under P2(b1)
                y0 = finish(ar2_o[0], "b0")

        y1 = finish(ar2_o[1], "b1")
        outT_sb = stgC.tile([DY, B], F32, tag="outT_sb", name="outT_sb")
        nc.vector.tensor_tensor(outT_sb, y0, y1, AluOpType.add)
        nc.sync.dma_start(outT_ap, outT_sb)
        p2c.release()


# =====================================================================
# host wrapper
# =====================================================================

_NC_CACHE = {}


def _get_nc():
    if "nc" not in _NC_CACHE:
        _NC_CACHE["nc"] = build_nc()
    return _NC_CACHE["nc"]


def _f32(a):
    return np.ascontiguousarray(np.asarray(a), dtype=np.float32)


def run(x, star_features, star_labels, features1, features2,
        labels_unique1, labels_unique2, label_distances1, label_distances2,
        W1, b1, W2, b2, label_indices1, label_indices2, trace=False):
    x = _f32(x)
    assert x.shape == (B, D) and star_features.shape == (N, D)
    nc = _get_nc()

    sf = _f32(star_features)
    sl = _f32(star_labels)
    f1 = _f32(features1)
    f2 = _f32(features2)
    li = [np.asarray(label_indices1).astype(np.int64),
          np.asarray(label_indices2).astype(np.int64)]
    uq = [_f32(labels_unique1), _f32(labels_unique2)]
    ld = [_f32(label_distances1), _f32(label_distances2)]
    Ws = [_f32(W1), _f32(W2)]
    bs = [_f32(b1), _f32(b2)]

    def bf16(a):
        return np.ascontiguousarray(a).astype(ml_dtypes.bfloat16)

    xTp = np.zeros((128, B), np.float32)
    xTp[0:D] = x.T
    common = {
        "xT": bf16(xTp),
        "ident": np.eye(128, dtype=np.float32),
        "onesr": np.ones((1, 128), np.float32),
        "onesc": np.ones((128, 1), ml_dtypes.bfloat16),
    }
    Ar = {}
    uqr2d = np.zeros((128, 256), np.float32)
    for j in (0, 1):
        # uqr rows 0:DY = -2 uq^T, row DY = |u_l|^2
        uqr = np.empty((DY + 1, L), np.float32)
        uqr[0:DY] = -2.0 * uq[j].T
        uqr[DY] = (uq[j].astype(np.float64) ** 2).sum(1).astype(np.float32)
        uqr2d[j * 64:j * 64 + DY + 1, j * 128:j * 128 + L] = uqr
    common["uqr2d"] = uqr2d
    for j in (0, 1):
        # Wb: rows 0:D = W, row D = b; col DY picks the ones row of xt
        Wb = np.zeros((D + 1, DY + 1), np.float32)
        Wb[0:D, 0:DY] = Ws[j]
        Wb[D, 0:DY] = bs[j].reshape(-1)
        Wb[D, DY] = 1.0
        common[f"Wb{j+1}"] = Wb
        # rank-RK SVD of ldist: ld ~ Arank @ Brank^T
        U_, S_, Vt_ = np.linalg.svd(ld[j].astype(np.float64))
        Arank = (U_[:, :RK] * S_[:RK]).astype(np.float32)     # [L, RK]
        Brank = Vt_[:RK, :].T.astype(np.float32)              # [L, RK]
        Ar[j] = Arank
        common[f"Bsc{j+1}"] = np.ascontiguousarray(
            (-ETA / 2.0) * Brank).astype(np.float32)

    in_maps = []
    for c in range(NCORES):
        r0, r1 = c * NSH_RAW, (c + 1) * NSH_RAW
        n_val = r1 - r0

        def padrows(a, width):
            out = np.zeros((NSH, width), np.float32)
            out[:n_val] = a[r0:r1]
            return out

        sfp = padrows(sf, D)
        f1p = padrows(f1, D)
        f2p = padrows(f2, D)
        slp = padrows(sl, DY)
        # f12t: per-tile [row, feat] blocks side by side
        f12 = np.concatenate([f1p, f2p], axis=1)                  # [NSH, 128]
        f12t = np.ascontiguousarray(
            f12.reshape(NT, 128, 128).transpose(1, 0, 2).reshape(128, NT * 128))
        # slo: labels + ones column per tile
        slo3 = np.zeros((NT, 128, DY + 1), np.float32)
        slo3[:, :, 0:DY] = slp.reshape(NT, 128, DY)
        slo3[:, :, DY] = 1.0
        slo = np.ascontiguousarray(
            slo3.transpose(1, 0, 2).reshape(128, NT * (DY + 1)))

        # exp biases -|row|^2 in [128, NT] layout, -1e30 kills pad rows
        def negn_of(a):
            nn = -(a.astype(np.float64) ** 2).sum(1).astype(np.float32)
            nn[n_val:] = -1e30
            return np.ascontiguousarray(nn.reshape(NT, 128).T)

        sfTp = np.zeros((128, NSH), np.float32)
        sfTp[0:D] = sfp.T
        m = {
            **common,
            "sfT": bf16(sfTp),
            "f12t": bf16(f12t),
            "slo": bf16(slo),
            "negnS": negn_of(sfp), "negn1": negn_of(f1p),
            "negn2": negn_of(f2p),
        }
        for j, fp in ((0, f1p), (1, f2p)):
            # fA rows 0:D = f^T, rows D:128 = A[lidx].T (pad rows zero)
            fA = np.zeros((128, NSH), np.float32)
            fA[0:D] = fp.T
            fA[D:D + RK, :n_val] = Ar[j][li[j][r0:r1], :].T
            m[f"fA{j+1}"] = bf16(fA)
        in_maps.append(m)

    res = run_bass_kernel_spmd(nc, in_maps, core_ids=list(range(NCORES)),
                               trace=trace)
    out = np.ascontiguousarray(res.results[0]["outT"].T).astype(np.float32)
    return out, res


def kernel(**inputs):
    out, _ = run(**inputs)
    return out

